# revision 52
# baseline (speedup 1.0000x reference)
"""Trainium2 Bass kernel for GQA causal self-attention (nn_CausalSelfAttention).

Model (hardcoded from the problem spec):
  B=2, T=2048, C=2048, n_head=32, n_kv=8, hs=64
  qkv = x @ w_attn.T + b_attn ; causal GQA attention ; y @ w_proj.T + b_proj

Sharding over 8 cores: core g handles batch b = g//4 and head-group grp = g%4
(8 q-heads, 2 kv-heads per core).  c_attn columns and c_proj rows are split
head-wise; the c_proj partial sums are reduced on the host (the "all-reduce").

Device layout notes:
 - qkv and c_proj GEMMs run in fp8e4m3 DoubleRow perf mode with a 3-term
   hi/lo error split (w_hi*x_hi + w_hi*x_lo + w_lo*x_hi): each DR matmul
   contracts 2 k-tiles at half the per-column cost, and the hi/lo split
   keeps quantization error ~0.2%.  Weights are pre-scaled by 64 on the
   host so fp8 sees O(1) magnitudes; the 1/64 is folded into the psum
   consumers.  x/w/wp ship as fp8 hi+lo pairs (same bytes as bf16).
 - Scores and PV stay bf16 (hs=64 contraction can't use DR; pt is produced
   on-device each block so an fp8 split of it would swamp the DVE).
 - All matmuls contract over the partition dim.  Host pre-transposes
   operands so no on-device transposes are needed.
 - Scores are computed K-stationary: S.T tile [tk, tq] = kT.T @ q, so
   softmax's P.T is directly the moving operand of the PV matmul.
 - exp without max-subtraction (scores are ~N(0,1); exp is safe in f32).
 - softmax denominator = ones-row appended to V (row 64 of the PV output).
 - normalization y = po * (16/den) is computed into an f32 staging tile,
   then split on the Pool engine into y_hi/y_lo fp8 pairs for the c_proj
   DR matmuls (scale 16 keeps y_lo out of fp8 subnormals; the 1/(64*16)
   is folded into the c_proj output scale).
 - q rows are stored interleaved ([h0,h4 | h1,h5 | h2,h6 | h3,h7] 64-row
   blocks) so each head's q/k share the same SBUF base partition (0 or 64).
 - heads are processed in pairs (h, h+4): their score matmuls use PE array
   rows 0:63 vs 64:127 (tile_position row groups); both land in one 2-bank
   psum tile so a single exp covers the pair.
 - block-causal: only tk-tiles <= the tq-tile are computed; in diagonal
   blocks the fully-masked leading columns are skipped in the matmul, exp,
   and PV (psum/pt slots are pre-zeroed so skipped regions stay finite).
 - emission is software-pipelined: projections for token-slice j+1 and
   c_proj for slice j-1 are round-robined between the attention units of
   slice j, keeping the PE busy while exps drain.
 - partial outputs leave the core as bf16 (halves output DMA); the host
   all-reduce accumulates in f32.
"""

import sys
import numpy as np
import ml_dtypes
from contextlib import ExitStack

for _p in ("/opt/trn_rl_repo", "/root/.axon_site/_ro/trn_rl_repo"):
    if _p not in sys.path:
        sys.path.append(_p)

import concourse.mybir as mybir
import concourse.tile as tile
from concourse import bacc
from concourse.bass_utils import run_bass_kernel_spmd

BF16 = mybir.dt.bfloat16
F32 = mybir.dt.float32
FP8 = mybir.dt.float8e4
NPBF16 = ml_dtypes.bfloat16
NPFP8 = ml_dtypes.float8_e4m3fn

B, T, C = 2, 2048, 2048
N_HEAD, N_KV, HS = 32, 8, 64
NE = 2048
N_CORES = 8
HL = 8          # q heads per core
KVL = 2         # kv heads per core
P = 128
TQ = 512        # tq tile (matmul moving width)
NJ = T // TQ    # 4 tq tiles
NT = T // P     # 16 token tiles
KC = C // P     # 16 contraction tiles over channels
KK = KC // 2    # 8 DR k-tile pairs
QROWS = HL * HS          # 512 local q rows
KROWS = KVL * HS         # 128 local k rows
WCOLS = QROWS + 2 * KROWS  # 768 local w_attn rows
WSCALE = 64.0   # fp8 pre-scale on w_attn / w_proj
YSCALE = 16.0   # fp8 pre-scale on normalized y
DR = mybir.MatmulPerfMode.DoubleRow

# position-block -> local head: q_sb m-tile mt rows [0:64]=head mt, [64:128]=head mt+4
Q_ORDER = [0, 4, 1, 5, 2, 6, 3, 7]

_CACHE = {}


def _build_program():
    nc = bacc.Bacc("TRN2", target_bir_lowering=False, debug=False)

    xh_d = nc.dram_tensor("xh", [P, KK, 2, T], FP8, kind="ExternalInput")
    xl_d = nc.dram_tensor("xl", [P, KK, 2, T], FP8, kind="ExternalInput")
    wh_d = nc.dram_tensor("wh", [P, KK, 2, WCOLS], FP8, kind="ExternalInput")
    wl_d = nc.dram_tensor("wl", [P, KK, 2, WCOLS], FP8, kind="ExternalInput")
    wph_d = nc.dram_tensor("wph", [P, 2, 2, C], FP8, kind="ExternalInput")
    wpl_d = nc.dram_tensor("wpl", [P, 2, 2, C], FP8, kind="ExternalInput")
    bq_d = nc.dram_tensor("bq", [4, P], F32, kind="ExternalInput")
    bk_d = nc.dram_tensor("bk", [1, P], F32, kind="ExternalInput")
    out_d = nc.dram_tensor("out", [T, C], BF16, kind="ExternalOutput")

    with tile.TileContext(nc) as tc:
        with ExitStack() as ctx:
            _emit(ctx, tc, nc, xh_d, xl_d, wh_d, wl_d, wph_d, wpl_d,
                  bq_d, bk_d, out_d)
    nc.compile()
    return nc


def _emit(ctx, tc, nc, xh_d, xl_d, wh_d, wl_d, wph_d, wpl_d, bq_d, bk_d, out_d):
    ExpF = mybir.ActivationFunctionType.Exp
    add = mybir.AluOpType.add
    mult = mybir.AluOpType.mult
    sub = mybir.AluOpType.subtract

    persist = ctx.enter_context(tc.tile_pool(name="persist", bufs=1))
    ppa = ctx.enter_context(tc.tile_pool(name="ppa", bufs=2, space="PSUM"))
    pps = ctx.enter_context(tc.tile_pool(name="pps", bufs=2, space="PSUM"))
    ppo = ctx.enter_context(tc.tile_pool(name="ppo", bufs=2, space="PSUM"))
    ptpool = ctx.enter_context(tc.tile_pool(name="pt", bufs=6))
    rcpool = ctx.enter_context(tc.tile_pool(name="rc", bufs=4))
    bcpool = ctx.enter_context(tc.tile_pool(name="bc", bufs=4))
    t1pool = ctx.enter_context(tc.tile_pool(name="t1", bufs=4))
    mkpool = ctx.enter_context(tc.tile_pool(name="mk", bufs=2))
    outpool = ctx.enter_context(tc.tile_pool(name="os", bufs=3))

    # ---- persistent SBUF tensors ----
    x8h = persist.tile([P, KK, 2, T], FP8, tag="x8h")
    x8l = persist.tile([P, KK, 2, T], FP8, tag="x8l")
    w8h = persist.tile([P, KK, 2, WCOLS], FP8, tag="w8h")
    w8l = persist.tile([P, KK, 2, WCOLS], FP8, tag="w8l")
    wp8h = persist.tile([P, 2, 2, C], FP8, tag="wp8h")
    wp8l = persist.tile([P, 2, 2, C], FP8, tag="wp8l")
    q_sb = persist.tile([P, 4 * T], BF16, tag="q")
    kT_sb = persist.tile([P, T], BF16, tag="k")
    v_sb = persist.tile([P, NT * 130], BF16, tag="v")
    y8h = persist.tile([P, 4, T], FP8, tag="y8h")
    y8l = persist.tile([P, 4, T], FP8, tag="y8l")
    bq_sb = persist.tile([P, 4], F32, tag="bq")
    bk_sb = persist.tile([P, 1], F32, tag="bk")
    # mask variants for diagonal blocks, doubled for the head-pair layout:
    # maskv[r][x, y] = 1 if (y mod 512)-x-128r >= 0 else 0
    maskv = [persist.tile([P, 2 * TQ], BF16, tag=f"mask{r}", name=f"mask{r}")
             for r in range(4)]

    # ---- input DMAs ----
    # HWDGE descriptor generation costs ~625ns per dma_start, so batch
    # transfers coarsely: weight halves + x chunk-0 halves first (the PE can
    # start on half 1 while half 2 streams), then whole-chunk x DMAs.
    for q in range(4):
        k0, k1 = q * (KK // 4), (q + 1) * (KK // 4)
        nc.sync.dma_start(w8h[:, k0:k1], wh_d.ap()[:, k0:k1])
        nc.sync.dma_start(x8h[:, k0:k1, :, 0:TQ], xh_d.ap()[:, k0:k1, :, 0:TQ])
    nc.sync.dma_start(bq_sb[:], bq_d.ap().rearrange("t p -> p t"))
    nc.sync.dma_start(bk_sb[:], bk_d.ap().rearrange("t p -> p t"))
    for half in range(2):
        k0, k1 = half * (KK // 2), (half + 1) * (KK // 2)
        nc.sync.dma_start(w8l[:, k0:k1], wl_d.ap()[:, k0:k1])
        nc.sync.dma_start(x8l[:, k0:k1, :, 0:TQ], xl_d.ap()[:, k0:k1, :, 0:TQ])
    # remaining x token chunks (proj(n) starts after chunk n)
    for n in range(1, NJ):
        nc.sync.dma_start(x8h[:, :, :, n * TQ:(n + 1) * TQ],
                          xh_d.ap()[:, :, :, n * TQ:(n + 1) * TQ])
        nc.sync.dma_start(x8l[:, :, :, n * TQ:(n + 1) * TQ],
                          xl_d.ap()[:, :, :, n * TQ:(n + 1) * TQ])
    nc.sync.dma_start(wp8h[:], wph_d.ap())
    nc.sync.dma_start(wp8l[:], wpl_d.ap())

    # ---- constants ----
    for r in range(4):
        mf = mkpool.tile([P, TQ], F32, tag="mf")
        nc.gpsimd.memset(mf[:], 1.0)
        nc.gpsimd.affine_select(
            out=mf[:], in_=mf[:], compare_op=mybir.AluOpType.is_ge,
            fill=0.0, base=-128 * r, pattern=[[1, TQ]], channel_multiplier=-1)
        nc.scalar.copy(maskv[r][:, 0:TQ], mf[:])
        nc.scalar.copy(maskv[r][:, TQ:2 * TQ], mf[:])
    nc.vector.memset(v_sb[:], 1.0)  # ones columns; data cols overwritten below
    # pre-zero the score psum slots: diagonal blocks are computed at reduced
    # width, so the masked-off region must hold finite values for exp()
    for w in range(2):
        pwarm = pps.tile([P, 2 * TQ], F32, tag="ps", name="pswarm")
        nc.vector.memset(pwarm[:], 0.0)
    for w in range(6):
        ptwarm = ptpool.tile([P, 2 * TQ], BF16, tag="pt", name="ptwarm")
        nc.gpsimd.memset(ptwarm[:], 0.0)

    # 3-term fp8 split order: x_lo last so its DMA can land late in window 0
    TERMS = ((w8h, x8h), (w8l, x8h), (w8h, x8l))

    # ---- work units ----
    # During the startup window (n == 0) the attention PSUM pools are idle,
    # so first-slice projection units borrow their banks for extra overlap.
    def _ppool(pool_sel):
        if pool_sel == 1:
            return pps, "ps"
        if pool_sel == 2:
            return ppo, "po"
        return ppa, "pa"

    def unit_q(n, mt, pool_sel=0):
        pool, tg = _ppool(pool_sel)
        ps = pool.tile([P, TQ], F32, tag=tg, name="psq")
        idx = 0
        for wt, xt in TERMS:
            for kk in range(KK):
                nc.tensor.matmul(
                    ps[:], wt[:, kk, :, mt * P:(mt + 1) * P],
                    xt[:, kk, :, n * TQ:(n + 1) * TQ],
                    start=(idx == 0), stop=(idx == 3 * KK - 1),
                    perf_mode=DR)
                idx += 1
                if idx % 4 == 0:
                    yield 427
        nc.vector.tensor_scalar(
            out=q_sb[:, mt * T + n * TQ: mt * T + (n + 1) * TQ],
            in0=ps[:], scalar1=1.0 / WSCALE, scalar2=bq_sb[:, mt:mt + 1],
            op0=mult, op1=add)

    def unit_k(n, pool_sel=0):
        pool, tg = _ppool(pool_sel)
        ps = pool.tile([P, TQ], F32, tag=tg, name="psk")
        idx = 0
        for wt, xt in TERMS:
            for kk in range(KK):
                nc.tensor.matmul(
                    ps[:], wt[:, kk, :, QROWS:QROWS + P],
                    xt[:, kk, :, n * TQ:(n + 1) * TQ],
                    start=(idx == 0), stop=(idx == 3 * KK - 1),
                    perf_mode=DR)
                idx += 1
                if idx % 4 == 0:
                    yield 427
        nc.vector.tensor_scalar(
            out=kT_sb[:, n * TQ:(n + 1) * TQ],
            in0=ps[:], scalar1=0.125 / WSCALE, scalar2=bk_sb[:, 0:1],
            op0=mult, op1=add)

    def unit_v(i, pool_sel=0):
        # v_sb tile i: [0:64]=kv0, 64=ones, [65:129]=kv1, 129=ones
        pool, tg = _ppool(pool_sel)
        ps = pool.tile([P, TQ], F32, tag=tg, name="psv")
        idx = 0
        for wt, xt in TERMS:
            for kk in range(KK):
                nc.tensor.matmul(
                    ps[:, 0:P], xt[:, kk, :, i * P:(i + 1) * P],
                    wt[:, kk, :, QROWS + P:QROWS + 2 * P],
                    start=(idx == 0), stop=(idx == 3 * KK - 1),
                    perf_mode=DR)
                idx += 1
            yield 214
        nc.vector.tensor_scalar(
            out=v_sb[:, i * 130: i * 130 + 64], in0=ps[:, 0:64],
            scalar1=1.0 / WSCALE, scalar2=None, op0=mult)
        nc.vector.tensor_scalar(
            out=v_sb[:, i * 130 + 65: i * 130 + 129], in0=ps[:, 64:128],
            scalar1=1.0 / WSCALE, scalar2=None, op0=mult)

    def unit_attn(j, hp):
        # processes the head pair (hp, hp+4): same q/y column tile `hp`,
        # head A on partitions 0:64 (kv0), head B on 64:128 (kv1).  Their
        # score matmuls are emitted adjacently so the PE runs them
        # concurrently on disjoint row-groups (tile_position 0 vs 64).
        # Generator: yields once per tk-block so filler matmuls can be woven
        # in at block granularity.  PV runs one block behind the scores so
        # the PE never sits in-order behind the exp it feeds.
        def pv(i, pt, c0):
            for h in (0, 1):
                nc.tensor.matmul(
                    po[h][:, c0:TQ],
                    v_sb[:, i * 130 + 65 * h: i * 130 + 65 * h + 65],
                    pt[:, h * TQ + c0:(h + 1) * TQ],
                    start=(i == 0), stop=(i == nb - 1))

        nb = 4 * (j + 1)   # tk tiles in play (block-causal)
        mt = hp
        qcol = mt * T + j * TQ
        po = {}
        po[0] = ppo.tile([65, TQ], F32, tag="po", name="poA")
        po[1] = ppo.tile([65, TQ], F32, tag="po", name="poB")
        prevs = []
        for i in range(nb):
            # ps cols [0:512] = head hp (array rows 0:64),
            #         [512:1024] = head hp+4 (array rows 64:128)
            ps = pps.tile([P, 2 * TQ], F32, tag="ps", name="pss")
            # diagonal blocks: cols < 128r are fully masked, skip them
            c0 = max(0, (i - 4 * j)) * P
            for h in (0, 1):
                rb = 64 * h
                nc.tensor.matmul(
                    ps[:, h * TQ + c0:(h + 1) * TQ],
                    kT_sb[rb:rb + 64, i * P:(i + 1) * P],
                    q_sb[rb:rb + 64, qcol + c0: qcol + TQ],
                    start=True, stop=True)
            pt = ptpool.tile([P, 2 * TQ], BF16, tag="pt", name="pt")
            nc.scalar.activation(pt[:, c0:2 * TQ], ps[:, c0:2 * TQ], ExpF)
            r = i - 4 * j
            if r >= 0:  # diagonal block: mask both head halves at once
                nc.vector.tensor_tensor(
                    out=pt[:, c0:2 * TQ], in0=pt[:, c0:2 * TQ],
                    in1=maskv[r][:, c0:2 * TQ], op=mult)
            yield              # filler chunk lands here, before PV(i-2)
            if len(prevs) == 5:
                pv(*prevs.pop(0))
            prevs.append((i, pt, c0))
        for pr in prevs:
            pv(*pr)
        # normalize + fp8 split: y = po[0:64] * (16/den); hi/lo fp8 for
        # the c_proj DR matmuls.  t1 staging frees the po bank; the
        # hi/lo quantization runs on the Pool engine.
        t1 = t1pool.tile([P, TQ], F32, tag="t1", name="t1")
        for h in (0, 1):
            rb = 64 * h
            rc = rcpool.tile([1, TQ], F32, tag="rc", name="rc")
            nc.vector.reciprocal(rc[:], po[h][64:65, :])
            # partition_broadcast only writes correctly at base 0, so
            # broadcast full-width and slice the needed half
            bc = bcpool.tile([P, TQ], F32, tag="bc", name="bc")
            nc.gpsimd.partition_broadcast(bc[:], rc[:])
            nc.vector.scalar_tensor_tensor(
                out=t1[rb:rb + 64, :], in0=po[h][0:64, :], scalar=YSCALE,
                in1=bc[rb:rb + 64, :], op0=mult, op1=mult)
            yh = y8h[rb:rb + 64, mt, j * TQ:(j + 1) * TQ]
            eng = (nc.vector if (j == NJ - 1 and hp == 3 and h == 1)
                   else nc.gpsimd)
            eng.tensor_copy(yh, t1[rb:rb + 64, :])
            eng.tensor_tensor(
                out=y8l[rb:rb + 64, mt, j * TQ:(j + 1) * TQ],
                in0=t1[rb:rb + 64, :], in1=yh, op=sub)

    def unit_cproj(j, ms, tail=False):
        os_t = outpool.tile([P, C], BF16, tag="os", name="os")
        pools = ((ppa, "pa"), (pps, "ps"), (ppo, "po")) if tail else ((ppa, "pa"),)
        for n in range(NJ):
            pool, tg = pools[n % len(pools)]
            pc = pool.tile([P, TQ], F32, tag=tg, name="pc")
            idx = 0
            for yt, wt in ((y8h, wp8h), (y8l, wp8h), (y8h, wp8l)):
                for kk in range(2):
                    nc.tensor.matmul(
                        pc[:],
                        yt[:, 2 * kk:2 * kk + 2,
                           j * TQ + ms * P: j * TQ + (ms + 1) * P],
                        wt[:, kk, :, n * TQ:(n + 1) * TQ],
                        start=(idx == 0), stop=(idx == 5),
                        perf_mode=DR)
                    idx += 1
            if j == NJ - 1 and (ms + n) % 2 == 1:
                # tail: ACT is idle, take every other psum drain off the DVE
                nc.scalar.mul(os_t[:, n * TQ:(n + 1) * TQ], pc[:],
                              1.0 / (WSCALE * YSCALE))
            else:
                nc.vector.tensor_scalar(
                    out=os_t[:, n * TQ:(n + 1) * TQ], in0=pc[:],
                    scalar1=1.0 / (WSCALE * YSCALE), scalar2=None, op0=mult)
            if j == NJ - 1:
                # spread tail DMA issue across idle sequencers so the last
                # transfers don't queue behind one engine's serial issue path
                deng = (nc.sync, nc.scalar, nc.gpsimd, nc.sync)[n]
                deng.dma_start(
                    out_d.ap()[j * TQ + ms * P: j * TQ + (ms + 1) * P,
                               n * TQ:(n + 1) * TQ],
                    os_t[:, n * TQ:(n + 1) * TQ])
            yield 640
        if j != NJ - 1:
            nc.sync.dma_start(
                out_d.ap()[j * TQ + ms * P: j * TQ + (ms + 1) * P, :], os_t[:])

    def proj_units(n):
        return ([unit_q(n, mt) for mt in range(4)] + [unit_k(n)]
                + [unit_v(i) for i in range(4 * n, 4 * n + 4)])

    def drain(g):
        for _ in g:
            pass

    # ---- software-pipelined emission ----
    # P(0) first (spread over all psum pools); then per j: the attention
    # pair generators yield once per tk-block and one filler step (a ~0.5us
    # chunk of a projection / c_proj unit) is woven in after each block, so
    # the in-order PE stream always has independent work between a score
    # matmul and the PV that waits on its exp.
    from collections import deque
    p0 = ([unit_k(0, pool_sel=0)]
          + [unit_q(0, mt, pool_sel=[0, 1, 1, 2][mt]) for mt in range(4)]
          + [unit_v(i, pool_sel=[1, 2, 1, 0][i]) for i in range(4)])
    for u in p0:
        drain(u)
    EST_PROJ = 5 * 3 * 854 + 4 * 3 * 214   # ns of one proj_units(n) batch
    EST_CPROJ = 4 * 640                     # ns of one unit_cproj(j, ms)
    for j in range(NJ):
        filler = deque()
        est = 0.0
        if j + 1 < NJ:
            filler.extend(proj_units(j + 1))
            est += EST_PROJ
        # c_proj work is deferred one extra window where possible so the
        # ACT-bound final windows get more PE filler
        if j == NJ - 1:
            filler.extend(unit_cproj(jj, ms) for jj in (j - 2, j - 1)
                          for ms in range(4))
            est += 8 * EST_CPROJ
        elif j - 1 >= 1:
            filler.extend(unit_cproj(j - 2, ms) for ms in range(4))
            est += 4 * EST_CPROJ
        # pace the filler evenly across the window's tk-blocks, reserving
        # ~20% to drain at the window end (covers the last pair's normalize
        # chain while the PE would otherwise idle)
        nblocks = 16 * (j + 1)
        budget = est / (nblocks + (36 if j == NJ - 1 else 8))
        credit = 0.0
        for hp in range(4):
            nb_seen = 0
            for _ in unit_attn(j, hp):
                credit += budget * (1.6 if nb_seen < 3 else 1.0)
                nb_seen += 1
                while filler and credit > 0:
                    try:
                        credit -= next(filler[0])
                    except StopIteration:
                        filler.popleft()
        for g in filler:
            drain(g)
    for ms in range(4):
        drain(unit_cproj(NJ - 1, ms, tail=True))
    # c_proj(0) ran in window 2 via the deferred schedule; nothing left here


def _split8(a):
    hi = a.astype(NPFP8)
    lo = (a - hi.astype(np.float32)).astype(NPFP8)
    return hi, lo


def _pack(a, cols):
    """[K, cols] (K=contraction) -> [128, K//256, 2, cols] DR slot layout."""
    kt = a.shape[0] // P
    return np.ascontiguousarray(
        a.reshape(kt, P, cols).transpose(1, 0, 2).reshape(P, kt // 2, 2, cols))


def _prep_inputs(x, w_attn, b_attn, w_proj):
    """Host-side shard + transpose + fp8 hi/lo split for each of the 8 cores."""
    in_maps = []
    for g in range(N_CORES):
        b, grp = divmod(g, 4)
        xT = np.ascontiguousarray(np.asarray(x[b], np.float32).T)
        xh, xl = _split8(xT)

        q_rows = []
        for lh in Q_ORDER:
            gh = HL * grp + lh
            q_rows.extend(range(HS * gh, HS * gh + HS))
        k0 = NE + KROWS * grp
        v0 = NE + N_KV * HS + KROWS * grp
        rows = q_rows + list(range(k0, k0 + KROWS)) + list(range(v0, v0 + KROWS))
        wT = np.ascontiguousarray(w_attn[rows, :].T) * WSCALE
        wh, wl = _split8(wT)

        cols = []
        for lh in Q_ORDER:
            gh = HL * grp + lh
            cols.extend(range(HS * gh, HS * gh + HS))
        wpT = np.ascontiguousarray(w_proj[:, cols].T) * WSCALE
        wph, wpl = _split8(wpT)

        bq = np.asarray(b_attn[q_rows], np.float32).reshape(4, P)
        bk = (np.asarray(b_attn[k0:k0 + KROWS], np.float32) / 8.0).reshape(1, P)

        in_maps.append({
            "xh": _pack(xh, T), "xl": _pack(xl, T),
            "wh": _pack(wh, WCOLS), "wl": _pack(wl, WCOLS),
            "wph": _pack(wph, C), "wpl": _pack(wpl, C),
            "bq": bq, "bk": bk})
    return in_maps


def get_nc():
    if "nc" not in _CACHE:
        _CACHE["nc"] = _build_program()
    return _CACHE["nc"]


def kernel(x, w_attn, b_attn, w_proj, b_proj):
    x = np.asarray(x, np.float32)
    w_attn = np.asarray(w_attn, np.float32)
    b_attn = np.asarray(b_attn, np.float32)
    w_proj = np.asarray(w_proj, np.float32)
    b_proj = np.asarray(b_proj, np.float32)

    nc = get_nc()
    in_maps = _prep_inputs(x, w_attn, b_attn, w_proj)
    res = run_bass_kernel_spmd(nc, in_maps, core_ids=list(range(N_CORES)))

    # host "all-reduce" over the 4 head-group cores per batch + bias folds
    bv = b_attn[NE + N_KV * HS:]                      # [512] v bias
    bv_full = np.repeat(bv.reshape(N_KV, HS), N_HEAD // N_KV, axis=0).reshape(-1)
    delta = bv_full @ w_proj.T + b_proj               # [2048]
    out = np.zeros((B, T, C), np.float32)
    for g in range(N_CORES):
        b = g // 4
        out[b] += np.asarray(res.results[g]["out"], np.float32)
    out += delta[None, None, :]
    return out


# revision 55
# speedup vs baseline: 1.0031x; 1.0031x over previous
"""Trainium2 Bass kernel for GQA causal self-attention (nn_CausalSelfAttention).

Model (hardcoded from the problem spec):
  B=2, T=2048, C=2048, n_head=32, n_kv=8, hs=64
  qkv = x @ w_attn.T + b_attn ; causal GQA attention ; y @ w_proj.T + b_proj

Sharding over 8 cores: core g handles batch b = g//4 and head-group grp = g%4
(8 q-heads, 2 kv-heads per core).  c_attn columns and c_proj rows are split
head-wise; the c_proj partial sums are reduced on the host (the "all-reduce").

Device layout notes:
 - qkv and c_proj GEMMs run in fp8e4m3 DoubleRow perf mode with a 3-term
   hi/lo error split (w_hi*x_hi + w_hi*x_lo + w_lo*x_hi): each DR matmul
   contracts 2 k-tiles at half the per-column cost, and the hi/lo split
   keeps quantization error ~0.2%.  Weights are pre-scaled by 64 on the
   host so fp8 sees O(1) magnitudes; the 1/64 is folded into the psum
   consumers.  x/w/wp ship as fp8 hi+lo pairs (same bytes as bf16).
 - Scores and PV stay bf16 (hs=64 contraction can't use DR; pt is produced
   on-device each block so an fp8 split of it would swamp the DVE).
 - All matmuls contract over the partition dim.  Host pre-transposes
   operands so no on-device transposes are needed.
 - Scores are computed K-stationary: S.T tile [tk, tq] = kT.T @ q, so
   softmax's P.T is directly the moving operand of the PV matmul.
 - exp without max-subtraction (scores are ~N(0,1); exp is safe in f32).
 - softmax denominator = ones-row appended to V (row 64 of the PV output).
 - normalization y = po * (16/den) is computed into an f32 staging tile,
   then split on the Pool engine into y_hi/y_lo fp8 pairs for the c_proj
   DR matmuls (scale 16 keeps y_lo out of fp8 subnormals; the 1/(64*16)
   is folded into the c_proj output scale).
 - q rows are stored interleaved ([h0,h4 | h1,h5 | h2,h6 | h3,h7] 64-row
   blocks) so each head's q/k share the same SBUF base partition (0 or 64).
 - heads are processed in pairs (h, h+4): their score matmuls use PE array
   rows 0:63 vs 64:127 (tile_position row groups); both land in one 2-bank
   psum tile so a single exp covers the pair.
 - block-causal: only tk-tiles <= the tq-tile are computed; in diagonal
   blocks the fully-masked leading columns are skipped in the matmul, exp,
   and PV (psum/pt slots are pre-zeroed so skipped regions stay finite).
 - emission is software-pipelined: projections for token-slice j+1 and
   c_proj for slice j-1 are round-robined between the attention units of
   slice j, keeping the PE busy while exps drain.
 - partial outputs leave the core as bf16 (halves output DMA); the host
   all-reduce accumulates in f32.
"""

import sys
import numpy as np
import ml_dtypes
from contextlib import ExitStack

for _p in ("/opt/trn_rl_repo", "/root/.axon_site/_ro/trn_rl_repo"):
    if _p not in sys.path:
        sys.path.append(_p)

import concourse.mybir as mybir
import concourse.tile as tile
from concourse import bacc
from concourse.bass_utils import run_bass_kernel_spmd

BF16 = mybir.dt.bfloat16
F32 = mybir.dt.float32
FP8 = mybir.dt.float8e4
NPBF16 = ml_dtypes.bfloat16
NPFP8 = ml_dtypes.float8_e4m3fn

B, T, C = 2, 2048, 2048
N_HEAD, N_KV, HS = 32, 8, 64
NE = 2048
N_CORES = 8
HL = 8          # q heads per core
KVL = 2         # kv heads per core
P = 128
TQ = 512        # tq tile (matmul moving width)
NJ = T // TQ    # 4 tq tiles
NT = T // P     # 16 token tiles
KC = C // P     # 16 contraction tiles over channels
KK = KC // 2    # 8 DR k-tile pairs
QROWS = HL * HS          # 512 local q rows
KROWS = KVL * HS         # 128 local k rows
WCOLS = QROWS + 2 * KROWS  # 768 local w_attn rows
WSCALE = 64.0   # fp8 pre-scale on w_attn / w_proj
YSCALE = 16.0   # fp8 pre-scale on normalized y
DR = mybir.MatmulPerfMode.DoubleRow

# position-block -> local head: q_sb m-tile mt rows [0:64]=head mt, [64:128]=head mt+4
Q_ORDER = [0, 4, 1, 5, 2, 6, 3, 7]

_CACHE = {}


def _build_program():
    nc = bacc.Bacc("TRN2", target_bir_lowering=False, debug=False)

    xh_d = nc.dram_tensor("xh", [P, KK, 2, T], FP8, kind="ExternalInput")
    xl_d = nc.dram_tensor("xl", [P, KK, 2, T], FP8, kind="ExternalInput")
    wh_d = nc.dram_tensor("wh", [P, KK, 2, WCOLS], FP8, kind="ExternalInput")
    wl_d = nc.dram_tensor("wl", [P, KK, 2, WCOLS], FP8, kind="ExternalInput")
    wph_d = nc.dram_tensor("wph", [P, 2, 2, C], FP8, kind="ExternalInput")
    wpl_d = nc.dram_tensor("wpl", [P, 2, 2, C], FP8, kind="ExternalInput")
    bq_d = nc.dram_tensor("bq", [4, P], F32, kind="ExternalInput")
    bk_d = nc.dram_tensor("bk", [1, P], F32, kind="ExternalInput")
    out_d = nc.dram_tensor("out", [T, C], BF16, kind="ExternalOutput")

    with tile.TileContext(nc) as tc:
        with ExitStack() as ctx:
            _emit(ctx, tc, nc, xh_d, xl_d, wh_d, wl_d, wph_d, wpl_d,
                  bq_d, bk_d, out_d)
    nc.compile()
    return nc


def _emit(ctx, tc, nc, xh_d, xl_d, wh_d, wl_d, wph_d, wpl_d, bq_d, bk_d, out_d):
    ExpF = mybir.ActivationFunctionType.Exp
    add = mybir.AluOpType.add
    mult = mybir.AluOpType.mult
    sub = mybir.AluOpType.subtract

    persist = ctx.enter_context(tc.tile_pool(name="persist", bufs=1))
    ppa = ctx.enter_context(tc.tile_pool(name="ppa", bufs=2, space="PSUM"))
    pps = ctx.enter_context(tc.tile_pool(name="pps", bufs=2, space="PSUM"))
    ppo = ctx.enter_context(tc.tile_pool(name="ppo", bufs=2, space="PSUM"))
    ptpool = ctx.enter_context(tc.tile_pool(name="pt", bufs=6))
    rcpool = ctx.enter_context(tc.tile_pool(name="rc", bufs=4))
    bcpool = ctx.enter_context(tc.tile_pool(name="bc", bufs=4))
    t1pool = ctx.enter_context(tc.tile_pool(name="t1", bufs=4))
    mkpool = ctx.enter_context(tc.tile_pool(name="mk", bufs=2))
    outpool = ctx.enter_context(tc.tile_pool(name="os", bufs=3))

    # ---- persistent SBUF tensors ----
    x8h = persist.tile([P, KK, 2, T], FP8, tag="x8h")
    x8l = persist.tile([P, KK, 2, T], FP8, tag="x8l")
    w8h = persist.tile([P, KK, 2, WCOLS], FP8, tag="w8h")
    w8l = persist.tile([P, KK, 2, WCOLS], FP8, tag="w8l")
    wp8h = persist.tile([P, 2, 2, C], FP8, tag="wp8h")
    wp8l = persist.tile([P, 2, 2, C], FP8, tag="wp8l")
    q_sb = persist.tile([P, 4 * T], BF16, tag="q")
    kT_sb = persist.tile([P, T], BF16, tag="k")
    v_sb = persist.tile([P, NT * 130], BF16, tag="v")
    y8h = persist.tile([P, 4, T], FP8, tag="y8h")
    y8l = persist.tile([P, 4, T], FP8, tag="y8l")
    bq_sb = persist.tile([P, 4], F32, tag="bq")
    bk_sb = persist.tile([P, 1], F32, tag="bk")
    # mask variants for diagonal blocks, doubled for the head-pair layout:
    # maskv[r][x, y] = 1 if (y mod 512)-x-128r >= 0 else 0
    maskv = [persist.tile([P, 2 * TQ], BF16, tag=f"mask{r}", name=f"mask{r}")
             for r in range(4)]

    # ---- input DMAs ----
    # HWDGE descriptor generation costs ~625ns per dma_start, so batch
    # transfers coarsely: weight halves + x chunk-0 halves first (the PE can
    # start on half 1 while half 2 streams), then whole-chunk x DMAs.
    for q in range(4):
        k0, k1 = q * (KK // 4), (q + 1) * (KK // 4)
        nc.sync.dma_start(w8h[:, k0:k1], wh_d.ap()[:, k0:k1])
        nc.sync.dma_start(x8h[:, k0:k1, :, 0:TQ], xh_d.ap()[:, k0:k1, :, 0:TQ])
    nc.sync.dma_start(bq_sb[:], bq_d.ap().rearrange("t p -> p t"))
    nc.sync.dma_start(bk_sb[:], bk_d.ap().rearrange("t p -> p t"))
    for half in range(2):
        k0, k1 = half * (KK // 2), (half + 1) * (KK // 2)
        nc.sync.dma_start(w8l[:, k0:k1], wl_d.ap()[:, k0:k1])
        nc.sync.dma_start(x8l[:, k0:k1, :, 0:TQ], xl_d.ap()[:, k0:k1, :, 0:TQ])
    # remaining x token chunks (proj(n) starts after chunk n)
    for n in range(1, NJ):
        nc.sync.dma_start(x8h[:, :, :, n * TQ:(n + 1) * TQ],
                          xh_d.ap()[:, :, :, n * TQ:(n + 1) * TQ])
        nc.sync.dma_start(x8l[:, :, :, n * TQ:(n + 1) * TQ],
                          xl_d.ap()[:, :, :, n * TQ:(n + 1) * TQ])
    nc.sync.dma_start(wp8h[:], wph_d.ap())
    nc.sync.dma_start(wp8l[:], wpl_d.ap())

    # ---- constants ----
    for r in range(4):
        mf = mkpool.tile([P, TQ], F32, tag="mf")
        nc.gpsimd.memset(mf[:], 1.0)
        nc.gpsimd.affine_select(
            out=mf[:], in_=mf[:], compare_op=mybir.AluOpType.is_ge,
            fill=0.0, base=-128 * r, pattern=[[1, TQ]], channel_multiplier=-1)
        nc.scalar.copy(maskv[r][:, 0:TQ], mf[:])
        nc.scalar.copy(maskv[r][:, TQ:2 * TQ], mf[:])
    nc.vector.memset(v_sb[:], 1.0)  # ones columns; data cols overwritten below
    # pre-zero the score psum slots: diagonal blocks are computed at reduced
    # width, so the masked-off region must hold finite values for exp()
    for w in range(2):
        pwarm = pps.tile([P, 2 * TQ], F32, tag="ps", name="pswarm")
        nc.vector.memset(pwarm[:], 0.0)
    for w in range(6):
        ptwarm = ptpool.tile([P, 2 * TQ], BF16, tag="pt", name="ptwarm")
        nc.gpsimd.memset(ptwarm[:], 0.0)

    # 3-term fp8 split order: x_lo last so its DMA can land late in window 0
    TERMS = ((w8h, x8h), (w8l, x8h), (w8h, x8l))

    # ---- work units ----
    # During the startup window (n == 0) the attention PSUM pools are idle,
    # so first-slice projection units borrow their banks for extra overlap.
    def _ppool(pool_sel):
        if pool_sel == 1:
            return pps, "ps"
        if pool_sel == 2:
            return ppo, "po"
        return ppa, "pa"

    def unit_q(n, mt, pool_sel=0):
        pool, tg = _ppool(pool_sel)
        ps = pool.tile([P, TQ], F32, tag=tg, name="psq")
        idx = 0
        for wt, xt in TERMS:
            for kk in range(KK):
                nc.tensor.matmul(
                    ps[:], wt[:, kk, :, mt * P:(mt + 1) * P],
                    xt[:, kk, :, n * TQ:(n + 1) * TQ],
                    start=(idx == 0), stop=(idx == 3 * KK - 1),
                    perf_mode=DR)
                idx += 1
                if idx % 4 == 0:
                    yield 427
        nc.vector.tensor_scalar(
            out=q_sb[:, mt * T + n * TQ: mt * T + (n + 1) * TQ],
            in0=ps[:], scalar1=1.0 / WSCALE, scalar2=bq_sb[:, mt:mt + 1],
            op0=mult, op1=add)

    def unit_k(n, pool_sel=0):
        pool, tg = _ppool(pool_sel)
        ps = pool.tile([P, TQ], F32, tag=tg, name="psk")
        idx = 0
        for wt, xt in TERMS:
            for kk in range(KK):
                nc.tensor.matmul(
                    ps[:], wt[:, kk, :, QROWS:QROWS + P],
                    xt[:, kk, :, n * TQ:(n + 1) * TQ],
                    start=(idx == 0), stop=(idx == 3 * KK - 1),
                    perf_mode=DR)
                idx += 1
                if idx % 4 == 0:
                    yield 427
        nc.vector.tensor_scalar(
            out=kT_sb[:, n * TQ:(n + 1) * TQ],
            in0=ps[:], scalar1=0.125 / WSCALE, scalar2=bk_sb[:, 0:1],
            op0=mult, op1=add)

    def unit_v(i, pool_sel=0):
        # v_sb tile i: [0:64]=kv0, 64=ones, [65:129]=kv1, 129=ones
        pool, tg = _ppool(pool_sel)
        ps = pool.tile([P, TQ], F32, tag=tg, name="psv")
        idx = 0
        for wt, xt in TERMS:
            for kk in range(KK):
                nc.tensor.matmul(
                    ps[:, 0:P], xt[:, kk, :, i * P:(i + 1) * P],
                    wt[:, kk, :, QROWS + P:QROWS + 2 * P],
                    start=(idx == 0), stop=(idx == 3 * KK - 1),
                    perf_mode=DR)
                idx += 1
            yield 214
        nc.vector.tensor_scalar(
            out=v_sb[:, i * 130: i * 130 + 64], in0=ps[:, 0:64],
            scalar1=1.0 / WSCALE, scalar2=None, op0=mult)
        nc.vector.tensor_scalar(
            out=v_sb[:, i * 130 + 65: i * 130 + 129], in0=ps[:, 64:128],
            scalar1=1.0 / WSCALE, scalar2=None, op0=mult)

    def unit_attn(j, hp):
        # processes the head pair (hp, hp+4): same q/y column tile `hp`,
        # head A on partitions 0:64 (kv0), head B on 64:128 (kv1).  Their
        # score matmuls are emitted adjacently so the PE runs them
        # concurrently on disjoint row-groups (tile_position 0 vs 64).
        # Generator: yields once per tk-block so filler matmuls can be woven
        # in at block granularity.  PV runs one block behind the scores so
        # the PE never sits in-order behind the exp it feeds.
        def pv(i, pt, c0):
            for h in (0, 1):
                nc.tensor.matmul(
                    po[h][:, c0:TQ],
                    v_sb[:, i * 130 + 65 * h: i * 130 + 65 * h + 65],
                    pt[:, h * TQ + c0:(h + 1) * TQ],
                    start=(i == 0), stop=(i == nb - 1))

        nb = 4 * (j + 1)   # tk tiles in play (block-causal)
        mt = hp
        qcol = mt * T + j * TQ
        po = {}
        po[0] = ppo.tile([65, TQ], F32, tag="po", name="poA")
        po[1] = ppo.tile([65, TQ], F32, tag="po", name="poB")
        prevs = []
        for i in range(nb):
            # ps cols [0:512] = head hp (array rows 0:64),
            #         [512:1024] = head hp+4 (array rows 64:128)
            ps = pps.tile([P, 2 * TQ], F32, tag="ps", name="pss")
            # diagonal blocks: cols < 128r are fully masked, skip them
            c0 = max(0, (i - 4 * j)) * P
            for h in (0, 1):
                rb = 64 * h
                nc.tensor.matmul(
                    ps[:, h * TQ + c0:(h + 1) * TQ],
                    kT_sb[rb:rb + 64, i * P:(i + 1) * P],
                    q_sb[rb:rb + 64, qcol + c0: qcol + TQ],
                    start=True, stop=True)
            pt = ptpool.tile([P, 2 * TQ], BF16, tag="pt", name="pt")
            nc.scalar.activation(pt[:, c0:2 * TQ], ps[:, c0:2 * TQ], ExpF)
            r = i - 4 * j
            if r >= 0:  # diagonal block: mask both head halves at once
                nc.vector.tensor_tensor(
                    out=pt[:, c0:2 * TQ], in0=pt[:, c0:2 * TQ],
                    in1=maskv[r][:, c0:2 * TQ], op=mult)
            yield              # filler chunk lands here, before PV(i-2)
            if len(prevs) == 5:
                pv(*prevs.pop(0))
            prevs.append((i, pt, c0))
        for pr in prevs:
            pv(*pr)
        # normalize + fp8 split: y = po[0:64] * (16/den); hi/lo fp8 for
        # the c_proj DR matmuls.  t1 staging frees the po bank; the
        # hi/lo quantization runs on the Pool engine.
        t1 = t1pool.tile([P, TQ], F32, tag="t1", name="t1")
        for h in (0, 1):
            rb = 64 * h
            rc = rcpool.tile([1, TQ], F32, tag="rc", name="rc")
            nc.vector.reciprocal(rc[:], po[h][64:65, :])
            # partition_broadcast only writes correctly at base 0, so
            # broadcast full-width and slice the needed half
            bc = bcpool.tile([P, TQ], F32, tag="bc", name="bc")
            nc.gpsimd.partition_broadcast(bc[:], rc[:])
            nc.vector.scalar_tensor_tensor(
                out=t1[rb:rb + 64, :], in0=po[h][0:64, :], scalar=YSCALE,
                in1=bc[rb:rb + 64, :], op0=mult, op1=mult)
            yh = y8h[rb:rb + 64, mt, j * TQ:(j + 1) * TQ]
            eng = (nc.vector if (j == NJ - 1 and hp == 3 and h == 1)
                   else nc.gpsimd)
            eng.tensor_copy(yh, t1[rb:rb + 64, :])
            eng.tensor_tensor(
                out=y8l[rb:rb + 64, mt, j * TQ:(j + 1) * TQ],
                in0=t1[rb:rb + 64, :], in1=yh, op=sub)

    def unit_cproj(j, ms, tail=False, late=None):
        os_t = outpool.tile([P, C], BF16, tag="os", name="os")
        rot = ((ppa, "pa"), (pps, "ps"), (ppo, "po"))
        for n in range(NJ):
            # after the window's attention ends (late flag), the attention
            # psum banks are free - rotate across all pools for deeper
            # drain pipelining
            pools = rot if (tail or (late is not None and late[0])) else rot[:1]
            pool, tg = pools[n % len(pools)]
            pc = pool.tile([P, TQ], F32, tag=tg, name="pc")
            idx = 0
            for yt, wt in ((y8h, wp8h), (y8l, wp8h), (y8h, wp8l)):
                for kk in range(2):
                    nc.tensor.matmul(
                        pc[:],
                        yt[:, 2 * kk:2 * kk + 2,
                           j * TQ + ms * P: j * TQ + (ms + 1) * P],
                        wt[:, kk, :, n * TQ:(n + 1) * TQ],
                        start=(idx == 0), stop=(idx == 5),
                        perf_mode=DR)
                    idx += 1
            if j == NJ - 1 and (ms + n) % 2 == 1:
                # tail: ACT is idle, take every other psum drain off the DVE
                nc.scalar.mul(os_t[:, n * TQ:(n + 1) * TQ], pc[:],
                              1.0 / (WSCALE * YSCALE))
            else:
                nc.vector.tensor_scalar(
                    out=os_t[:, n * TQ:(n + 1) * TQ], in0=pc[:],
                    scalar1=1.0 / (WSCALE * YSCALE), scalar2=None, op0=mult)
            if j == NJ - 1:
                # spread tail DMA issue across idle sequencers so the last
                # transfers don't queue behind one engine's serial issue path
                deng = (nc.sync, nc.scalar, nc.gpsimd, nc.sync)[n]
                deng.dma_start(
                    out_d.ap()[j * TQ + ms * P: j * TQ + (ms + 1) * P,
                               n * TQ:(n + 1) * TQ],
                    os_t[:, n * TQ:(n + 1) * TQ])
            yield 640
        if j != NJ - 1:
            nc.sync.dma_start(
                out_d.ap()[j * TQ + ms * P: j * TQ + (ms + 1) * P, :], os_t[:])

    def proj_units(n):
        return ([unit_q(n, mt) for mt in range(4)] + [unit_k(n)]
                + [unit_v(i) for i in range(4 * n, 4 * n + 4)])

    def drain(g):
        for _ in g:
            pass

    # ---- software-pipelined emission ----
    # P(0) first (spread over all psum pools); then per j: the attention
    # pair generators yield once per tk-block and one filler step (a ~0.5us
    # chunk of a projection / c_proj unit) is woven in after each block, so
    # the in-order PE stream always has independent work between a score
    # matmul and the PV that waits on its exp.
    from collections import deque
    p0 = ([unit_k(0, pool_sel=0)]
          + [unit_q(0, mt, pool_sel=[0, 1, 1, 2][mt]) for mt in range(4)]
          + [unit_v(i, pool_sel=[1, 2, 1, 0][i]) for i in range(4)])
    for u in p0:
        drain(u)
    EST_PROJ = 5 * 3 * 854 + 4 * 3 * 214   # ns of one proj_units(n) batch
    EST_CPROJ = 4 * 640                     # ns of one unit_cproj(j, ms)
    for j in range(NJ):
        filler = deque()
        est = 0.0
        if j + 1 < NJ:
            filler.extend(proj_units(j + 1))
            est += EST_PROJ
        # c_proj work is deferred one extra window where possible so the
        # ACT-bound final windows get more PE filler
        late = [False]
        if j == NJ - 1:
            filler.extend(unit_cproj(jj, ms, late=late)
                          for jj in (j - 2, j - 1) for ms in range(4))
            est += 8 * EST_CPROJ
        elif j - 1 >= 1:
            filler.extend(unit_cproj(j - 2, ms, late=late) for ms in range(4))
            est += 4 * EST_CPROJ
        # pace the filler evenly across the window's tk-blocks, reserving
        # ~20% to drain at the window end (covers the last pair's normalize
        # chain while the PE would otherwise idle)
        nblocks = 16 * (j + 1)
        budget = est / (nblocks + (36 if j == NJ - 1 else 8))
        credit = 0.0
        for hp in range(4):
            nb_seen = 0
            for _ in unit_attn(j, hp):
                credit += budget * (1.6 if nb_seen < 3 else 1.0)
                nb_seen += 1
                while filler and credit > 0:
                    try:
                        credit -= next(filler[0])
                    except StopIteration:
                        filler.popleft()
        late[0] = True
        for g in filler:
            drain(g)
    for ms in range(4):
        drain(unit_cproj(NJ - 1, ms, tail=True))
    # c_proj(0) ran in window 2 via the deferred schedule; nothing left here


def _split8(a):
    hi = a.astype(NPFP8)
    lo = (a - hi.astype(np.float32)).astype(NPFP8)
    return hi, lo


def _pack(a, cols):
    """[K, cols] (K=contraction) -> [128, K//256, 2, cols] DR slot layout."""
    kt = a.shape[0] // P
    return np.ascontiguousarray(
        a.reshape(kt, P, cols).transpose(1, 0, 2).reshape(P, kt // 2, 2, cols))


def _prep_inputs(x, w_attn, b_attn, w_proj):
    """Host-side shard + transpose + fp8 hi/lo split for each of the 8 cores."""
    in_maps = []
    for g in range(N_CORES):
        b, grp = divmod(g, 4)
        xT = np.ascontiguousarray(np.asarray(x[b], np.float32).T)
        xh, xl = _split8(xT)

        q_rows = []
        for lh in Q_ORDER:
            gh = HL * grp + lh
            q_rows.extend(range(HS * gh, HS * gh + HS))
        k0 = NE + KROWS * grp
        v0 = NE + N_KV * HS + KROWS * grp
        rows = q_rows + list(range(k0, k0 + KROWS)) + list(range(v0, v0 + KROWS))
        wT = np.ascontiguousarray(w_attn[rows, :].T) * WSCALE
        wh, wl = _split8(wT)

        cols = []
        for lh in Q_ORDER:
            gh = HL * grp + lh
            cols.extend(range(HS * gh, HS * gh + HS))
        wpT = np.ascontiguousarray(w_proj[:, cols].T) * WSCALE
        wph, wpl = _split8(wpT)

        bq = np.asarray(b_attn[q_rows], np.float32).reshape(4, P)
        bk = (np.asarray(b_attn[k0:k0 + KROWS], np.float32) / 8.0).reshape(1, P)

        in_maps.append({
            "xh": _pack(xh, T), "xl": _pack(xl, T),
            "wh": _pack(wh, WCOLS), "wl": _pack(wl, WCOLS),
            "wph": _pack(wph, C), "wpl": _pack(wpl, C),
            "bq": bq, "bk": bk})
    return in_maps


def get_nc():
    if "nc" not in _CACHE:
        _CACHE["nc"] = _build_program()
    return _CACHE["nc"]


def kernel(x, w_attn, b_attn, w_proj, b_proj):
    x = np.asarray(x, np.float32)
    w_attn = np.asarray(w_attn, np.float32)
    b_attn = np.asarray(b_attn, np.float32)
    w_proj = np.asarray(w_proj, np.float32)
    b_proj = np.asarray(b_proj, np.float32)

    nc = get_nc()
    in_maps = _prep_inputs(x, w_attn, b_attn, w_proj)
    res = run_bass_kernel_spmd(nc, in_maps, core_ids=list(range(N_CORES)))

    # host "all-reduce" over the 4 head-group cores per batch + bias folds
    bv = b_attn[NE + N_KV * HS:]                      # [512] v bias
    bv_full = np.repeat(bv.reshape(N_KV, HS), N_HEAD // N_KV, axis=0).reshape(-1)
    delta = bv_full @ w_proj.T + b_proj               # [2048]
    out = np.zeros((B, T, C), np.float32)
    for g in range(N_CORES):
        b = g // 4
        out[b] += np.asarray(res.results[g]["out"], np.float32)
    out += delta[None, None, :]
    return out
